# revision 18
# baseline (speedup 1.0000x reference)
"""2-layer GCN encoder on 8 Trainium2 NeuronCores.

Strategy (dst-tile-sharded graph parallel, 2 SPMD launches):
  By linearity, the per-layer weight matmul commutes with aggregation:
    conv(x) = dinv . A^T (dinv . x) @ W, so each launch gathers raw table
  rows (dma_gather, 4 SWDGE queues in parallel), segment-sums them via fp8
  one-hot indicator matmuls into PSUM per 128-dst tile, and applies the
  dense W at psum eviction (scale dinv -> PE transpose -> @W -> act).
    Launch A: table = dinv.x (bf16), evict chain ends in relu -> g1 rows.
    Launch B: table = g1 (bf16), evict chain ends in f32 output rows.
  Self-loop contributions are dense (identity-matmul of the core's own
  table rows) instead of gathered. dst tiles are assigned to cores by
  matching group-count profiles, which shrinks the shared SPMD max-over-
  cores group structure and balances gather volume.
Host relays the g table between launches (the all-to-all) and precomputes
the edge structure (sort, windows, indicators) in numpy from edge_index.
"""

import sys

sys.path.insert(0, "/opt/trn_rl_repo")

import ml_dtypes
import numpy as np

from concourse import bacc, bass, library_config, mybir, tile
from concourse.bass_utils import run_bass_kernel_spmd

N_NODES = 100000
IN_C = 128
H2 = 128  # 2*hid
HID = 64
NCORES = 8
P = 128
NTG = 784  # global 128-node tiles (node space padded to 100352)
NT = NTG // NCORES  # 98 schedule slots per core
NPAD = NTG * P
WIN = 25000
NW = 4
TBATCH = 4  # slots per psum batch (one accumulator per psum bank)
MAXG = 32  # groups per dma_gather call (4096 idx)
NMSG = 5  # rotating gather staging buffers
NQ = 4  # SWDGE queues (Q7 core pairs) used round-robin
SCRATCH = 32768

BF16 = ml_dtypes.bfloat16
FP8 = ml_dtypes.float8_e4m3


# ---------------------------------------------------------------- host prep
def _prepare(edge_index):
    src = np.asarray(edge_index[0], dtype=np.int64)
    dst = np.asarray(edge_index[1], dtype=np.int64)
    deg = np.bincount(dst, minlength=N_NODES).astype(np.float32) + 1.0
    dinv = np.zeros(NPAD, dtype=np.float32)
    dinv[:N_NODES] = 1.0 / np.sqrt(deg)

    tile_g = dst // P  # [E] global tile of each edge
    win = src // WIN
    key = tile_g * NW + win
    cnt_tw = np.bincount(key, minlength=NTG * NW).reshape(NTG, NW)
    ceil_tw = -(-cnt_tw // P)  # [NTG, NW]

    # slot assignment: sort tiles by ceil profile; chunks of 8 share a slot
    order = np.lexsort(
        (ceil_tw[:, 3], ceil_tw[:, 2], ceil_tw[:, 1], ceil_tw[:, 0],
         ceil_tw.sum(1))
    )
    slot_tiles = order.reshape(NT, 8)  # [NT, 8] tile ids per slot
    # within a slot, assign heaviest tile to least-loaded core
    T = np.zeros((NCORES, NT), dtype=np.int64)  # core, slot -> global tile
    load = np.zeros(NCORES, dtype=np.int64)
    for s in range(NT):
        tiles = slot_tiles[s]
        sz = cnt_tw[tiles].sum(1)
        avail = list(range(NCORES))
        for t_id in tiles[np.argsort(-sz)]:
            k = avail[int(np.argmin(load[avail]))]
            avail.remove(k)
            T[k, s] = t_id
            load[k] += cnt_tw[t_id].sum()

    G = ceil_tw[slot_tiles].max(axis=1)  # [NT, NW] shared group counts

    # batches and global group order: for b: for w: for s in b: G[s,w] groups
    batches = [range(b, min(b + TBATCH, NT)) for b in range(0, NT, TBATCH)]
    group_slot = []
    calls_by_batch = []  # per batch: list of (w, goff, gc)
    for slots_b in batches:
        bcalls = []
        for w in range(NW):
            seg = []
            for s in slots_b:
                seg += [s] * int(G[s, w])
            goff = len(group_slot)
            for cs in range(0, len(seg), MAXG):
                bcalls.append((w, goff + cs, min(MAXG, len(seg) - cs)))
            group_slot += seg
        calls_by_batch.append(bcalls)
    group_slot = np.array(group_slot, dtype=np.int64)
    GT = len(group_slot)
    calls = [c for bc in calls_by_batch for c in bc]

    # group start offset of each (s, w) run
    g_of_run = np.full((NT, NW), -1, dtype=np.int64)
    gi = 0
    for slots_b in batches:
        for w in range(NW):
            for s in slots_b:
                g_of_run[s, w] = gi
                gi += G[s, w]
    assert gi == GT

    # stop flags: last group of each slot (identity matmul opens the acc)
    stop_flag = np.zeros(GT, dtype=bool)
    id_stop = np.zeros(NT, dtype=bool)
    for s in range(NT):
        gs = np.nonzero(group_slot == s)[0]
        if len(gs):
            stop_flag[gs[-1]] = True
        else:
            id_stop[s] = True

    # per-core idx16 / indicators / dinv / own-row order
    per_core = []
    for k in range(NCORES):
        tile_of_edge = np.full(NTG, -1, dtype=np.int64)
        tile_of_edge[T[k]] = np.arange(NT)
        m = tile_of_edge[tile_g] >= 0
        es, ed, ew = src[m], dst[m], win[m]
        slot = tile_of_edge[tile_g[m]]
        o = np.lexsort((ed, ew, slot))
        es, ed, ew, slot = es[o], ed[o], ew[o], slot[o]
        cnt = cnt_tw[T[k]]  # [NT, NW]
        run_off = np.concatenate([[0], np.cumsum(cnt.reshape(-1))])[:-1].reshape(
            NT, NW
        )

        idx_local = np.zeros(GT * P, dtype=np.int16)  # pads stay 0 (safe row)
        ind = np.zeros((P, GT * P), dtype=FP8)
        for s in range(NT):
            for w in range(NW):
                n = cnt[s, w]
                if n == 0:
                    continue
                pos0 = g_of_run[s, w] * P
                o0 = run_off[s, w]
                sl = slice(pos0, pos0 + n)
                idx_local[sl] = (es[o0 : o0 + n] - w * WIN).astype(np.int16)
                cols = ed[o0 : o0 + n] - T[k, s] * P
                pos = np.arange(pos0, pos0 + n)
                ind[pos % P, (pos // P) * P + cols] = FP8(1.0)
        arr = idx_local.reshape(GT * P // 16, 16).T  # [16, GT*8]
        idx16 = np.concatenate([arr] * 8, axis=0).copy()  # [128, GT*8]
        dv = dinv[(T[k][None, :] * P + np.arange(P)[:, None])].astype(
            np.float32
        )  # [128, NT]
        per_core.append(dict(idx16=idx16, ind=ind, dinv=dv))

    return dict(
        G=G, GT=GT, calls=calls, calls_by_batch=calls_by_batch,
        group_slot=group_slot, stop_flag=stop_flag,
        id_stop=id_stop, dinv=dinv, T=T, per_core=per_core,
    )


# --------------------------------------------------------------- arrange
def _arrange_own(table_pad, T):
    """[NPAD, 128] -> per-core [128, NT, 128] own-tile rows (node = s*128+p)."""
    outs = []
    for k in range(NCORES):
        rows = table_pad[(T[k][:, None] * P + np.arange(P)[None, :]).reshape(-1)]
        outs.append(
            np.ascontiguousarray(rows.reshape(NT, P, -1).transpose(1, 0, 2))
        )
    return outs


# ---------------------------------------------------------------- launches
def _build(plan, layer, reps=1):
    """layer=1: table=xs, evict relu(@W1) -> bf16 g1 rows.
    layer=2: table=g1, evict @W2 -> f32 out rows.
    reps>1 repeats the (idempotent) schedule in-program for slope timing."""
    GT = plan["GT"]
    calls = plan["calls"]
    gslot = plan["group_slot"]
    stopf = plan["stop_flag"]
    id_stop = plan["id_stop"]
    feat_out = H2 if layer == 1 else HID

    nc = bacc.Bacc(
        name=f"gcn_l{layer}",
        dynamic_dma_scratch_size=SCRATCH,
        num_swdge_queues=NQ,
    )
    gtab = nc.dram_tensor(
        "gtab", [N_NODES, H2], mybir.dt.bfloat16, kind="ExternalInput"
    )
    idx = nc.dram_tensor("idx", [128, GT * 8], mybir.dt.int16, kind="ExternalInput")
    indt = nc.dram_tensor("indt", [P, GT * P], mybir.dt.float8e4, kind="ExternalInput")
    own = nc.dram_tensor("own", [P, NT, H2], mybir.dt.bfloat16, kind="ExternalInput")
    dv = nc.dram_tensor("dv", [P, NT], mybir.dt.float32, kind="ExternalInput")
    wmat = nc.dram_tensor(
        "wmat", [H2, feat_out], mybir.dt.bfloat16, kind="ExternalInput"
    )
    idn = nc.dram_tensor("idn", [P, P], mybir.dt.bfloat16, kind="ExternalInput")
    idf = nc.dram_tensor("idf", [P, P], mybir.dt.float8e4, kind="ExternalInput")
    odt = mybir.dt.bfloat16 if layer == 1 else mybir.dt.float32
    outT = nc.dram_tensor("outT", [P, NT, feat_out], odt, kind="ExternalOutput")

    with tile.TileContext(nc) as tc:
        with (
            tc.tile_pool(name="sbuf", bufs=1) as pool,
            tc.tile_pool(name="psum", bufs=1, space="PSUM") as psum,
        ):
            nc.gpsimd.load_library(library_config.mlp)
            idx_s = pool.tile([128, GT * 8], mybir.dt.int16)
            dv_s = pool.tile([P, NT], mybir.dt.float32)
            own_s = pool.tile([P, NT, H2], mybir.dt.bfloat16)
            w_s = pool.tile([H2, feat_out], mybir.dt.bfloat16)
            idn_s = pool.tile([P, P], mybir.dt.bfloat16)
            idf_s = pool.tile([P, P], mybir.dt.float8e4)
            nc.sync.dma_start(out=idx_s[:], in_=idx[:])
            nc.sync.dma_start(out=dv_s[:], in_=dv[:])
            nc.sync.dma_start(out=own_s[:], in_=own[:])
            nc.sync.dma_start(out=w_s[:], in_=wmat[:])
            nc.sync.dma_start(out=idn_s[:], in_=idn[:])
            nc.sync.dma_start(out=idf_s[:], in_=idf[:])
            out_s = pool.tile([P, NT, feat_out], odt)

            msgbufs = [
                pool.tile([P, MAXG, H2], mybir.dt.bfloat16, name=f"msg{j}")
                for j in range(NMSG)
            ]
            indbufs = [
                pool.tile([P, MAXG * P], mybir.dt.float8e4, name=f"ind{j}")
                for j in range(NMSG)
            ]

            accs = {}

            def acc_for(s):
                if s not in accs:
                    accs[s] = psum.tile(
                        [P, H2], mybir.dt.float32,
                        name=f"acc{rep}_{s}", tag=f"acc{s % TBATCH}", bufs=1,
                    )
                return accs[s]

            def evict(s):
                acc = accs.pop(s)
                t0 = pool.tile(
                    [P, H2], mybir.dt.bfloat16, name=f"t0_{rep}_{s}", tag="t0", bufs=3
                )
                nc.scalar.activation(
                    out=t0[:], in_=acc[:],
                    func=mybir.ActivationFunctionType.Copy,
                    scale=dv_s[:, s : s + 1],
                )
                tp = psum.tile(
                    [P, P], mybir.dt.bfloat16, name=f"tp{rep}_{s}", tag="tp", bufs=2
                )
                nc.tensor.transpose(out=tp[:], in_=t0[:], identity=idn_s[:])
                t0T = pool.tile(
                    [P, P], mybir.dt.bfloat16, name=f"t0T_{rep}_{s}", tag="t0T", bufs=3
                )
                nc.scalar.copy(out=t0T[:], in_=tp[:])
                hp = psum.tile(
                    [P, feat_out], mybir.dt.float32,
                    name=f"hp{rep}_{s}", tag="mo", bufs=2,
                )
                nc.tensor.matmul(
                    out=hp[:], lhsT=t0T[:], rhs=w_s[:], start=True, stop=True
                )
                if layer == 1:
                    nc.scalar.activation(
                        out=out_s[:, s, :], in_=hp[:],
                        func=mybir.ActivationFunctionType.Relu,
                        scale=dv_s[:, s : s + 1],
                    )
                else:
                    nc.scalar.copy(out=out_s[:, s, :], in_=hp[:])

            # schedule: per batch, identities first, then calls/groups
            for rep in range(reps):
              cj = 0
              for bi, slots_b in enumerate(
                range(b0, min(b0 + TBATCH, NT))
                for b0 in range(0, NT, TBATCH)
              ):
                for s in slots_b:
                    nc.tensor.matmul(
                        out=acc_for(s)[:],
                        lhsT=idf_s[:],
                        rhs=own_s[:, s, :],
                        start=True,
                        stop=bool(id_stop[s]),
                        skip_group_check=True,
                    )
                    if id_stop[s]:
                        evict(s)
                for w, goff, gc in plan["calls_by_batch"][bi]:
                    msg = msgbufs[cj % NMSG]
                    ind_c = indbufs[cj % NMSG]
                    nc.sync.dma_start(
                        out=ind_c[:, : gc * P],
                        in_=indt[:, goff * P : (goff + gc) * P],
                    )
                    nidx = gc * P
                    nc.gpsimd.dma_gather(
                        out_ap=msg[:, :gc, :],
                        in_ap=gtab[w * WIN : (w + 1) * WIN, :],
                        idxs_ap=idx_s[:, goff * 8 : (goff + gc) * 8],
                        num_idxs=nidx,
                        num_idxs_reg=nidx,
                        elem_size=H2,
                        single_packet=(nidx <= 1024),
                        queue_num=cj % NQ,
                    )
                    for gl in range(gc):
                        g = goff + gl
                        s = int(gslot[g])
                        nc.tensor.matmul(
                            out=acc_for(s)[:],
                            lhsT=ind_c[:, gl * P : (gl + 1) * P],
                            rhs=msg[:, gl, :],
                            start=False,
                            stop=bool(stopf[g]),
                            skip_group_check=True,
                        )
                        if stopf[g]:
                            evict(s)
                    cj += 1
            assert cj == len(calls) and not accs, (cj, len(calls), accs.keys())
            nc.sync.dma_start(out=outT[:], in_=out_s[:])
    nc.compile()
    return nc


# ---------------------------------------------------------------- kernel
def kernel(x, edge_index, W1, b1, W2, b2):
    x = np.asarray(x)
    W1 = np.asarray(W1)
    b1 = np.asarray(b1)
    W2 = np.asarray(W2)
    b2 = np.asarray(b2)
    assert not b1.any() and not b2.any(), "nonzero bias unsupported fast path"

    plan = _prepare(np.asarray(edge_index))
    dinv = plan["dinv"]  # [NPAD]
    T = plan["T"]

    xs_pad = np.zeros((NPAD, IN_C), dtype=np.float32)
    xs_pad[:N_NODES] = x * dinv[:N_NODES, None]
    xs_pad = xs_pad.astype(BF16)
    own1 = _arrange_own(xs_pad, T)

    idn = np.eye(P, dtype=BF16)
    idf = np.eye(P, dtype=FP8)
    w1b = W1.astype(BF16)

    nc1 = _build(plan, 1)
    in1 = []
    for k in range(NCORES):
        pc = plan["per_core"][k]
        in1.append(
            {"gtab": xs_pad[:N_NODES], "idx": pc["idx16"], "indt": pc["ind"],
             "own": own1[k], "dv": pc["dinv"], "wmat": w1b, "idn": idn,
             "idf": idf}
        )
    r1 = run_bass_kernel_spmd(nc1, in1, core_ids=list(range(NCORES)))

    g1_pad = np.zeros((NPAD, H2), dtype=BF16)
    for k in range(NCORES):
        o = r1.results[k]["outT"]  # [P, NT, H2]
        g1_pad[(T[k][:, None] * P + np.arange(P)[None, :]).reshape(-1)] = (
            o.transpose(1, 0, 2).reshape(NT * P, H2)
        )
    own2 = _arrange_own(g1_pad, T)

    nc2 = _build(plan, 2)
    w2b = W2.astype(BF16)
    in2 = []
    for k in range(NCORES):
        pc = plan["per_core"][k]
        in2.append(
            {"gtab": g1_pad[:N_NODES], "idx": pc["idx16"], "indt": pc["ind"],
             "own": own2[k], "dv": pc["dinv"], "wmat": w2b, "idn": idn,
             "idf": idf}
        )
    r2 = run_bass_kernel_spmd(nc2, in2, core_ids=list(range(NCORES)))

    out = np.zeros((N_NODES, HID), dtype=np.float32)
    out_pad = np.zeros((NPAD, HID), dtype=np.float32)
    for k in range(NCORES):
        o = r2.results[k]["outT"]  # [P, NT, HID] f32
        out_pad[(T[k][:, None] * P + np.arange(P)[None, :]).reshape(-1)] = (
            o.transpose(1, 0, 2).reshape(NT * P, HID)
        )
    out[:] = out_pad[:N_NODES]
    return out


# revision 19
# speedup vs baseline: 1.1638x; 1.1638x over previous
"""2-layer GCN encoder on 8 Trainium2 NeuronCores.

Strategy (dst-tile-sharded graph parallel, 2 SPMD launches):
  By linearity, the per-layer weight matmul commutes with aggregation:
    conv(x) = dinv . A^T (dinv . x) @ W, so each launch gathers raw table
  rows (dma_gather, 4 SWDGE queues in parallel), segment-sums them via fp8
  one-hot indicator matmuls into PSUM per 128-dst tile, and applies the
  dense W at psum eviction (scale dinv -> PE transpose -> @W -> act).
    Launch A: table = dinv.x (bf16), evict chain ends in relu -> g1 rows.
    Launch B: table = g1 (bf16), evict chain ends in f32 output rows.
  Self-loop contributions are dense (identity-matmul of the core's own
  table rows) instead of gathered. dst tiles are assigned to cores by
  matching group-count profiles, which shrinks the shared SPMD max-over-
  cores group structure and balances gather volume.
Host relays the g table between launches (the all-to-all) and precomputes
the edge structure (sort, windows, indicators) in numpy from edge_index.
"""

import sys

sys.path.insert(0, "/opt/trn_rl_repo")

import ml_dtypes
import numpy as np

from concourse import bacc, bass, library_config, mybir, tile
from concourse.bass_utils import run_bass_kernel_spmd

N_NODES = 100000
IN_C = 128
H2 = 128  # 2*hid
HID = 64
NCORES = 8
P = 128
NTG = 784  # global 128-node tiles (node space padded to 100352)
NT = NTG // NCORES  # 98 schedule slots per core
NPAD = NTG * P
WIN = 25000
NW = 4
TBATCH = 4  # slots per psum batch (one accumulator per psum bank)
MAXG = 32  # groups per dma_gather call (4096 idx)
NMSG = 6  # rotating gather staging buffers
NQ = 4  # SWDGE queues (Q7 core pairs) used round-robin
SCRATCH = 32768

BF16 = ml_dtypes.bfloat16
FP8 = ml_dtypes.float8_e4m3


# ---------------------------------------------------------------- host prep
def _prepare(edge_index):
    src = np.asarray(edge_index[0], dtype=np.int64)
    dst = np.asarray(edge_index[1], dtype=np.int64)
    deg = np.bincount(dst, minlength=N_NODES).astype(np.float32) + 1.0
    dinv = np.zeros(NPAD, dtype=np.float32)
    dinv[:N_NODES] = 1.0 / np.sqrt(deg)

    tile_g = dst // P  # [E] global tile of each edge
    win = src // WIN
    key = tile_g * NW + win
    cnt_tw = np.bincount(key, minlength=NTG * NW).reshape(NTG, NW)
    ceil_tw = -(-cnt_tw // P)  # [NTG, NW]

    # slot assignment: sort tiles by ceil profile; chunks of 8 share a slot
    order = np.lexsort(
        (ceil_tw[:, 3], ceil_tw[:, 2], ceil_tw[:, 1], ceil_tw[:, 0],
         ceil_tw.sum(1))
    )
    slot_tiles = order.reshape(NT, 8)  # [NT, 8] tile ids per slot
    # within a slot, assign heaviest tile to least-loaded core
    T = np.zeros((NCORES, NT), dtype=np.int64)  # core, slot -> global tile
    load = np.zeros(NCORES, dtype=np.int64)
    for s in range(NT):
        tiles = slot_tiles[s]
        sz = cnt_tw[tiles].sum(1)
        avail = list(range(NCORES))
        for t_id in tiles[np.argsort(-sz)]:
            k = avail[int(np.argmin(load[avail]))]
            avail.remove(k)
            T[k, s] = t_id
            load[k] += cnt_tw[t_id].sum()

    G = ceil_tw[slot_tiles].max(axis=1)  # [NT, NW] shared group counts

    # batches and global group order: for b: for w: for s in b: G[s,w] groups
    batches = [range(b, min(b + TBATCH, NT)) for b in range(0, NT, TBATCH)]
    group_slot = []
    calls_by_batch = []  # per batch: list of (w, goff, gc)
    for slots_b in batches:
        bcalls = []
        for w in range(NW):
            seg = []
            for s in slots_b:
                seg += [s] * int(G[s, w])
            goff = len(group_slot)
            for cs in range(0, len(seg), MAXG):
                bcalls.append((w, goff + cs, min(MAXG, len(seg) - cs)))
            group_slot += seg
        calls_by_batch.append(bcalls)
    group_slot = np.array(group_slot, dtype=np.int64)
    GT = len(group_slot)
    calls = [c for bc in calls_by_batch for c in bc]

    # group start offset of each (s, w) run
    g_of_run = np.full((NT, NW), -1, dtype=np.int64)
    gi = 0
    for slots_b in batches:
        for w in range(NW):
            for s in slots_b:
                g_of_run[s, w] = gi
                gi += G[s, w]
    assert gi == GT

    # stop flags: last group of each slot (identity matmul opens the acc)
    stop_flag = np.zeros(GT, dtype=bool)
    id_stop = np.zeros(NT, dtype=bool)
    for s in range(NT):
        gs = np.nonzero(group_slot == s)[0]
        if len(gs):
            stop_flag[gs[-1]] = True
        else:
            id_stop[s] = True

    # per-core idx16 / indicators / dinv / own-row order
    per_core = []
    for k in range(NCORES):
        tile_of_edge = np.full(NTG, -1, dtype=np.int64)
        tile_of_edge[T[k]] = np.arange(NT)
        m = tile_of_edge[tile_g] >= 0
        es, ed, ew = src[m], dst[m], win[m]
        slot = tile_of_edge[tile_g[m]]
        o = np.lexsort((ed, ew, slot))
        es, ed, ew, slot = es[o], ed[o], ew[o], slot[o]
        cnt = cnt_tw[T[k]]  # [NT, NW]
        run_off = np.concatenate([[0], np.cumsum(cnt.reshape(-1))])[:-1].reshape(
            NT, NW
        )

        idx_local = np.zeros(GT * P, dtype=np.int16)  # pads stay 0 (safe row)
        ind = np.zeros((P, GT * P), dtype=FP8)
        for s in range(NT):
            for w in range(NW):
                n = cnt[s, w]
                if n == 0:
                    continue
                pos0 = g_of_run[s, w] * P
                o0 = run_off[s, w]
                sl = slice(pos0, pos0 + n)
                idx_local[sl] = (es[o0 : o0 + n] - w * WIN).astype(np.int16)
                cols = ed[o0 : o0 + n] - T[k, s] * P
                pos = np.arange(pos0, pos0 + n)
                ind[pos % P, (pos // P) * P + cols] = FP8(1.0)
        arr = idx_local.reshape(GT * P // 16, 16).T  # [16, GT*8]
        idx16 = np.concatenate([arr] * 8, axis=0).copy()  # [128, GT*8]
        dv = dinv[(T[k][None, :] * P + np.arange(P)[:, None])].astype(
            np.float32
        )  # [128, NT]
        per_core.append(dict(idx16=idx16, ind=ind, dinv=dv))

    return dict(
        G=G, GT=GT, calls=calls, calls_by_batch=calls_by_batch,
        group_slot=group_slot, stop_flag=stop_flag,
        id_stop=id_stop, dinv=dinv, T=T, per_core=per_core,
    )


# --------------------------------------------------------------- arrange
def _arrange_own(table_pad, T):
    """[NPAD, 128] -> per-core [128, NT, 128] own-tile rows (node = s*128+p)."""
    outs = []
    for k in range(NCORES):
        rows = table_pad[(T[k][:, None] * P + np.arange(P)[None, :]).reshape(-1)]
        outs.append(
            np.ascontiguousarray(rows.reshape(NT, P, -1).transpose(1, 0, 2))
        )
    return outs


# ---------------------------------------------------------------- launches
def _build(plan, layer, reps=1):
    """layer=1: table=xs, evict relu(@W1) -> bf16 g1 rows.
    layer=2: table=g1, evict @W2 -> f32 out rows.
    reps>1 repeats the (idempotent) schedule in-program for slope timing."""
    GT = plan["GT"]
    calls = plan["calls"]
    gslot = plan["group_slot"]
    stopf = plan["stop_flag"]
    id_stop = plan["id_stop"]
    feat_out = H2 if layer == 1 else HID

    nc = bacc.Bacc(
        name=f"gcn_l{layer}",
        dynamic_dma_scratch_size=SCRATCH,
        num_swdge_queues=NQ,
    )
    gtab = nc.dram_tensor(
        "gtab", [N_NODES, H2], mybir.dt.bfloat16, kind="ExternalInput"
    )
    idx = nc.dram_tensor("idx", [128, GT * 8], mybir.dt.int16, kind="ExternalInput")
    indt = nc.dram_tensor("indt", [P, GT * P], mybir.dt.float8e4, kind="ExternalInput")
    own = nc.dram_tensor("own", [P, NT, H2], mybir.dt.bfloat16, kind="ExternalInput")
    dv = nc.dram_tensor("dv", [P, NT], mybir.dt.float32, kind="ExternalInput")
    wmat = nc.dram_tensor(
        "wmat", [H2, feat_out], mybir.dt.bfloat16, kind="ExternalInput"
    )
    idn = nc.dram_tensor("idn", [P, P], mybir.dt.bfloat16, kind="ExternalInput")
    idf = nc.dram_tensor("idf", [P, P], mybir.dt.float8e4, kind="ExternalInput")
    odt = mybir.dt.bfloat16 if layer == 1 else mybir.dt.float32
    outT = nc.dram_tensor("outT", [P, NT, feat_out], odt, kind="ExternalOutput")

    with tile.TileContext(nc) as tc:
        with (
            tc.tile_pool(name="sbuf", bufs=1) as pool,
            tc.tile_pool(name="psum", bufs=1, space="PSUM") as psum,
        ):
            nc.gpsimd.load_library(library_config.mlp)
            idx_s = pool.tile([128, GT * 8], mybir.dt.int16)
            dv_s = pool.tile([P, NT], mybir.dt.float32)
            own_s = pool.tile([P, NT, H2], mybir.dt.bfloat16)
            w_s = pool.tile([H2, feat_out], mybir.dt.bfloat16)
            idn_s = pool.tile([P, P], mybir.dt.bfloat16)
            idf_s = pool.tile([P, P], mybir.dt.float8e4)
            out_s = pool.tile([P, NT, feat_out], odt)

            msgbufs = [
                pool.tile([P, MAXG, H2], mybir.dt.bfloat16, name=f"msg{j}")
                for j in range(NMSG)
            ]
            indbufs = [
                pool.tile([P, MAXG * P], mybir.dt.float8e4, name=f"ind{j}")
                for j in range(NMSG)
            ]

            accs = {}

            def acc_for(s):
                if s not in accs:
                    accs[s] = psum.tile(
                        [P, H2], mybir.dt.float32,
                        name=f"acc{rep}_{s}", tag=f"acc{s % TBATCH}", bufs=1,
                    )
                return accs[s]

            def evict(s):
                acc = accs.pop(s)
                t0 = pool.tile(
                    [P, H2], mybir.dt.bfloat16, name=f"t0_{rep}_{s}", tag="t0", bufs=3
                )
                nc.scalar.activation(
                    out=t0[:], in_=acc[:],
                    func=mybir.ActivationFunctionType.Copy,
                    scale=dv_s[:, s : s + 1],
                )
                tp = psum.tile(
                    [P, P], mybir.dt.bfloat16, name=f"tp{rep}_{s}", tag="tp", bufs=2
                )
                nc.tensor.transpose(out=tp[:], in_=t0[:], identity=idn_s[:])
                t0T = pool.tile(
                    [P, P], mybir.dt.bfloat16, name=f"t0T_{rep}_{s}", tag="t0T", bufs=3
                )
                nc.scalar.copy(out=t0T[:], in_=tp[:])
                hp = psum.tile(
                    [P, feat_out], mybir.dt.float32,
                    name=f"hp{rep}_{s}", tag="mo", bufs=2,
                )
                nc.tensor.matmul(
                    out=hp[:], lhsT=t0T[:], rhs=w_s[:], start=True, stop=True
                )
                if layer == 1:
                    nc.scalar.activation(
                        out=out_s[:, s, :], in_=hp[:],
                        func=mybir.ActivationFunctionType.Relu,
                        scale=dv_s[:, s : s + 1],
                    )
                else:
                    nc.scalar.copy(out=out_s[:, s, :], in_=hp[:])

            # schedule: per batch, identities first, then calls/groups
            for rep in range(reps):
              nc.sync.dma_start(out=idx_s[:], in_=idx[:])
              nc.sync.dma_start(out=dv_s[:], in_=dv[:])
              nc.sync.dma_start(out=own_s[:], in_=own[:])
              nc.sync.dma_start(out=w_s[:], in_=wmat[:])
              nc.sync.dma_start(out=idn_s[:], in_=idn[:])
              nc.sync.dma_start(out=idf_s[:], in_=idf[:])
              cj = 0
              for bi, slots_b in enumerate(
                range(b0, min(b0 + TBATCH, NT))
                for b0 in range(0, NT, TBATCH)
              ):
                for s in slots_b:
                    nc.tensor.matmul(
                        out=acc_for(s)[:],
                        lhsT=idf_s[:],
                        rhs=own_s[:, s, :],
                        start=True,
                        stop=bool(id_stop[s]),
                        skip_group_check=True,
                    )
                    if id_stop[s]:
                        evict(s)
                for w, goff, gc in plan["calls_by_batch"][bi]:
                    msg = msgbufs[cj % NMSG]
                    ind_c = indbufs[cj % NMSG]
                    nc.sync.dma_start(
                        out=ind_c[:, : gc * P],
                        in_=indt[:, goff * P : (goff + gc) * P],
                    )
                    nidx = gc * P
                    nc.gpsimd.dma_gather(
                        out_ap=msg[:, :gc, :],
                        in_ap=gtab[w * WIN : (w + 1) * WIN, :],
                        idxs_ap=idx_s[:, goff * 8 : (goff + gc) * 8],
                        num_idxs=nidx,
                        num_idxs_reg=nidx,
                        elem_size=H2,
                        single_packet=(nidx <= 1024),
                        queue_num=cj % NQ,
                    )
                    for gl in range(gc):
                        g = goff + gl
                        s = int(gslot[g])
                        nc.tensor.matmul(
                            out=acc_for(s)[:],
                            lhsT=ind_c[:, gl * P : (gl + 1) * P],
                            rhs=msg[:, gl, :],
                            start=False,
                            stop=bool(stopf[g]),
                            skip_group_check=True,
                        )
                        if stopf[g]:
                            evict(s)
                    cj += 1
              assert cj == len(calls) and not accs, (cj, len(calls), accs.keys())
              nc.sync.dma_start(out=outT[:], in_=out_s[:])
    nc.compile()
    return nc


# ---------------------------------------------------------------- kernel
def kernel(x, edge_index, W1, b1, W2, b2):
    x = np.asarray(x)
    W1 = np.asarray(W1)
    b1 = np.asarray(b1)
    W2 = np.asarray(W2)
    b2 = np.asarray(b2)
    assert not b1.any() and not b2.any(), "nonzero bias unsupported fast path"

    plan = _prepare(np.asarray(edge_index))
    dinv = plan["dinv"]  # [NPAD]
    T = plan["T"]

    xs_pad = np.zeros((NPAD, IN_C), dtype=np.float32)
    xs_pad[:N_NODES] = x * dinv[:N_NODES, None]
    xs_pad = xs_pad.astype(BF16)
    own1 = _arrange_own(xs_pad, T)

    idn = np.eye(P, dtype=BF16)
    idf = np.eye(P, dtype=FP8)
    w1b = W1.astype(BF16)

    nc1 = _build(plan, 1)
    in1 = []
    for k in range(NCORES):
        pc = plan["per_core"][k]
        in1.append(
            {"gtab": xs_pad[:N_NODES], "idx": pc["idx16"], "indt": pc["ind"],
             "own": own1[k], "dv": pc["dinv"], "wmat": w1b, "idn": idn,
             "idf": idf}
        )
    r1 = run_bass_kernel_spmd(nc1, in1, core_ids=list(range(NCORES)))

    g1_pad = np.zeros((NPAD, H2), dtype=BF16)
    for k in range(NCORES):
        o = r1.results[k]["outT"]  # [P, NT, H2]
        g1_pad[(T[k][:, None] * P + np.arange(P)[None, :]).reshape(-1)] = (
            o.transpose(1, 0, 2).reshape(NT * P, H2)
        )
    own2 = _arrange_own(g1_pad, T)

    nc2 = _build(plan, 2)
    w2b = W2.astype(BF16)
    in2 = []
    for k in range(NCORES):
        pc = plan["per_core"][k]
        in2.append(
            {"gtab": g1_pad[:N_NODES], "idx": pc["idx16"], "indt": pc["ind"],
             "own": own2[k], "dv": pc["dinv"], "wmat": w2b, "idn": idn,
             "idf": idf}
        )
    r2 = run_bass_kernel_spmd(nc2, in2, core_ids=list(range(NCORES)))

    out = np.zeros((N_NODES, HID), dtype=np.float32)
    out_pad = np.zeros((NPAD, HID), dtype=np.float32)
    for k in range(NCORES):
        o = r2.results[k]["outT"]  # [P, NT, HID] f32
        out_pad[(T[k][:, None] * P + np.arange(P)[None, :]).reshape(-1)] = (
            o.transpose(1, 0, 2).reshape(NT * P, HID)
        )
    out[:] = out_pad[:N_NODES]
    return out
